# revision 8
# baseline (speedup 1.0000x reference)
"""Trainium2 Bass kernel: transformer encoder layer (S=4096,B=2,D=512,H=8,F=2048),
causal attention + RoPE, distributed over 8 NeuronCores.

Sharding (SPMD, v2 — AllToAll resharding, no AllGather):
  - LN1+RoPE: sequence-parallel (core c owns s in [512c, 512(c+1)), both batches)
  - QKV for ALL heads on LOCAL tokens (contraction over D, local xr/xn kept in SBUF)
  - AllToAll(qkv, 1.5MB/rank, per batch) -> core c gets head c's q,k,v for ALL s
  - causal attention: head-parallel (core c owns head c, full S, per batch)
  - AllToAll(attn, 0.5MB/rank, per batch) -> each core gets all heads for its tokens
  - out_proj + residual + LN2 + FFN: token-parallel (core c owns its s-slice)
LayerNorm affine params are folded into downstream weights host-side.
Softmax denominators come free from a ones-column appended to V; the
denominator reciprocal is broadcast across partitions with a PE outer product.
"""
import numpy as np
import ml_dtypes
from contextlib import ExitStack

import concourse.bass as bass
import concourse.tile as tile
from concourse import bacc, mybir
from concourse.bass_utils import run_bass_kernel_spmd
from concourse.masks import make_identity

F32 = mybir.dt.float32
F32R = mybir.dt.float32r
BF16 = mybir.dt.bfloat16
AF = mybir.ActivationFunctionType
ALU = mybir.AluOpType

S, B, D, H, Dh, F = 4096, 2, 512, 8, 64, 2048
W = 8                    # cores
SL = S // W              # 512 s-positions per core
TL = SL * B              # 1024 local tokens
EPS = 1e-5
SCALE = 1.0 / float(np.sqrt(Dh))  # 0.125

NT = TL // 128           # 8 local token tiles
NK = D // 128            # 4 contraction chunks over D
NF = F // 128            # 16 chunks over F
NS = S // 128            # 32 key tiles per batch

_NC_CACHE = {}
_PHASE_MARKS = []
_GELU_OVERRIDE = None  # set to AF.Identity in sim tests (CoreSim lacks Gelu)


def _layer_norm_stats(nc, pool, x_t, eps_sb):
    """Returns (rstd [128,1], negmean_rstd [128,1]) for rows of x_t."""
    stats = pool.tile([128, 6], F32, tag="st")
    nc.vector.bn_stats(out=stats, in_=x_t)
    mv = pool.tile([128, 2], F32, tag="mv")
    nc.vector.bn_aggr(out=mv, in_=stats)
    sd = pool.tile([128, 1], F32, tag="sd")
    nc.scalar.activation(out=sd, in_=mv[:, 1:2], func=AF.Sqrt, bias=eps_sb)
    rstd = pool.tile([128, 1], F32, tag="rs")
    nc.vector.reciprocal(out=rstd, in_=sd)
    nm = pool.tile([128, 1], F32, tag="nm")
    nc.vector.tensor_mul(nm, mv[:, 0:1], rstd)
    nc.vector.tensor_scalar_mul(nm, nm, -1.0)
    return rstd, nm


def _build_nc(flags, n_reps=1):
    """flags = (has_ropeb, has_bq, has_bk, has_bv, has_bo, has_b2)

    n_reps > 1 builds a timing variant with the body unrolled n_reps times
    (same I/O, idempotent) so device time can be read off the slope.
    """
    import os as _os
    has_ropeb, has_bq, has_bk, has_bv, has_bo, has_b2 = flags
    skip_cc = bool(int(_os.environ.get("K_SKIP_CC", "0")))
    max_phase = int(_os.environ.get("K_MAX_PHASE", "7"))
    nc = bacc.Bacc("TRN2", target_bir_lowering=False, debug=False, num_devices=W)

    # ---- I/O ----
    src_loc = nc.dram_tensor("src_loc", [TL, D], F32, kind="ExternalInput")
    cosw = nc.dram_tensor("cosw", [SL, D], F32, kind="ExternalInput")
    rotw = nc.dram_tensor("rotw", [SL, D], F32, kind="ExternalInput")
    ropeb = nc.dram_tensor("ropeb", [SL, D], F32, kind="ExternalInput") if has_ropeb else None
    wqk_t = nc.dram_tensor("wqk_t", [D, 2 * D], BF16, kind="ExternalInput")
    wv_t = nc.dram_tensor("wv_t", [D, D], BF16, kind="ExternalInput")
    bqk = nc.dram_tensor("bqk", [2 * Dh], F32, kind="ExternalInput")
    bvh = nc.dram_tensor("bvh", [Dh], F32, kind="ExternalInput")
    wo_t = nc.dram_tensor("wo_t", [D, D], BF16, kind="ExternalInput")
    bo = nc.dram_tensor("bo", [D], F32, kind="ExternalInput")
    w1_t = nc.dram_tensor("w1_t", [D, F], BF16, kind="ExternalInput")
    b1p = nc.dram_tensor("b1p", [F], F32, kind="ExternalInput")
    w2_t = nc.dram_tensor("w2_t", [F, D], BF16, kind="ExternalInput")
    b2 = nc.dram_tensor("b2", [D], F32, kind="ExternalInput")
    out_loc = nc.dram_tensor("out_loc", [TL, D], F32, kind="ExternalOutput")

    with tile.TileContext(nc) as tc:
      for _rep in range(n_reps):
       with ExitStack() as top:
        dram = top.enter_context(tc.tile_pool(name="dram", bufs=1, space="DRAM"))
        consts = top.enter_context(tc.tile_pool(name="consts", bufs=1))
        wpool = top.enter_context(tc.tile_pool(name="weights", bufs=1))
        act = top.enter_context(tc.tile_pool(name="act", bufs=1))

        # ---------- constants ----------
        ident = consts.tile([128, 128], F32)
        make_identity(nc, ident)
        # causal diag masks: masks[:, j, q] = 1.0 if q >= k + j*128 else 0.0
        masks = consts.tile([128, 4, 512], BF16)
        for j in range(4):
            nc.gpsimd.memset(masks[:, j, :], 1.0)
            nc.gpsimd.affine_select(
                out=masks[:, j, :], in_=masks[:, j, :],
                compare_op=ALU.is_ge, fill=0.0,
                base=-j * 128, channel_multiplier=-1, pattern=[[1, 512]],
            )
        eps_sb = consts.tile([128, 1], F32)
        nc.vector.memset(eps_sb, EPS)
        ones_row = consts.tile([1, Dh], F32)
        nc.vector.memset(ones_row, 1.0)
        bq_sb = consts.tile([Dh, 1], F32)
        nc.sync.dma_start(out=bq_sb, in_=bqk[0:Dh, None])
        bk_sb = consts.tile([Dh, 1], F32)
        nc.sync.dma_start(out=bk_sb, in_=bqk[Dh:2 * Dh, None])
        bv_bc = consts.tile([128, Dh], F32)
        if has_bv:
            bv_row = consts.tile([1, Dh], F32)
            nc.sync.dma_start(out=bv_row, in_=bvh[None, :])
            nc.gpsimd.partition_broadcast(bv_bc, bv_row)
        bo_bc = consts.tile([128, D], F32)
        if has_bo:
            bo_row = consts.tile([1, D], F32)
            nc.sync.dma_start(out=bo_row, in_=bo[None, :])
            nc.gpsimd.partition_broadcast(bo_bc, bo_row)
        b2_bc = consts.tile([128, D], F32)
        if has_b2:
            b2_row = consts.tile([1, D], F32)
            nc.sync.dma_start(out=b2_row, in_=b2[None, :])
            nc.gpsimd.partition_broadcast(b2_bc, b2_row)
        b1_sb = consts.tile([128, NF], F32)
        nc.sync.dma_start(out=b1_sb, in_=b1p.rearrange("(m p) -> p m", p=128))

        # ---------- persistent weights (SBUF) ----------
        wqk_sb = wpool.tile([128, NK, 2 * D], BF16)
        nc.sync.dma_start(out=wqk_sb, in_=wqk_t.rearrange("(k p) m -> p k m", p=128))
        wv_sb = wpool.tile([128, NK, D], BF16)
        nc.sync.dma_start(out=wv_sb, in_=wv_t.rearrange("(k p) m -> p k m", p=128))
        wo_sb = wpool.tile([128, NK, D], BF16)
        nc.sync.dma_start(out=wo_sb, in_=wo_t.rearrange("(k p) n -> p k n", p=128))
        w1_sb = wpool.tile([128, NK, F], BF16)
        nc.sync.dma_start(out=w1_sb, in_=w1_t.rearrange("(k p) n -> p k n", p=128))
        w2_sb = wpool.tile([128, NF, D], BF16)
        nc.sync.dma_start(out=w2_sb, in_=w2_t.rearrange("(m p) n -> p m n", p=128))

        # ---------- persistent activations (SBUF) ----------
        qT = act.tile([Dh, B, S], BF16)        # my head's q, D-major
        kT = act.tile([Dh, B, S], BF16)        # my head's k, D-major
        vS = act.tile([128, B, NS, 65], BF16)  # token-major V + ones column
        nc.vector.memset(vS[:, :, :, 64:65], 1.0)
        attnT = act.tile([Dh, B, S], BF16)
        out1 = act.tile([128, NT, D], F32)     # post-attention residual stream
        yT = act.tile([128, NK, TL], BF16)     # LN2 output, D-major

        # collective buffers (per batch)
        cc3_in = [dram.tile([W, 3, Dh * SL], BF16, name=f"cc3_in_{b}") for b in range(B)]
        cc3_out = [dram.tile([W, 3, Dh * SL], BF16, name=f"cc3_out_{b}")
                   for b in range(B)]
        cc2_in = [dram.tile([W, Dh, SL], BF16, name=f"cc2_in_{b}") for b in range(B)]
        cc2_out = [dram.tile([W, Dh, SL], BF16, name=f"cc2_out_{b}")
                   for b in range(B)]

        _PHASE_MARKS.append(("consts", nc.next_id()))

        # ====== P1 + P2a per batch: LN1+RoPE, local QKV (all heads), A2A ======
        with ExitStack() as ctx:
          xln = ctx.enter_context(tc.tile_pool(name="xln", bufs=1))
          sb = ctx.enter_context(tc.tile_pool(name="p1", bufs=3))
          small = ctx.enter_context(tc.tile_pool(name="p1s", bufs=4))
          trps = ctx.enter_context(tc.tile_pool(name="p1ps", bufs=4, space="PSUM"))
          qv = ctx.enter_context(tc.tile_pool(name="p2", bufs=3))
          qvps = ctx.enter_context(tc.tile_pool(name="p2ps", bufs=2, space="PSUM"))
          xrT = xln.tile([128, NK, TL], BF16)   # D-major LN1+RoPE output
          xnT = xln.tile([128, NK, TL], BF16)   # D-major LN1 output
          for bb in range(B):
            # ---- P1: LN + RoPE + transpose for this batch's 4 tiles ----
            for tt in range(NT // B):
                t = bb * (NT // B) + tt
                s_t = sb.tile([128, D], F32, tag="s")
                nc.sync.dma_start(out=s_t, in_=src_loc[t * 128:(t + 1) * 128, :])
                rstd, nm = _layer_norm_stats(nc, small, s_t, eps_sb)
                xn_t = sb.tile([128, D], F32, tag="xn")
                nc.vector.tensor_scalar(
                    out=xn_t, in0=s_t, scalar1=rstd, scalar2=nm,
                    op0=ALU.mult, op1=ALU.add,
                )
                # RoPE (ln1 affine folded into cosw/rotw/ropeb host-side)
                cosw_t = sb.tile([128, D], F32, tag="cw")
                nc.sync.dma_start(out=cosw_t, in_=cosw[tt * 128:(tt + 1) * 128, :])
                rotw_t = sb.tile([128, D], F32, tag="rw")
                nc.sync.dma_start(out=rotw_t, in_=rotw[tt * 128:(tt + 1) * 128, :])
                xr_t = sb.tile([128, D], F32, tag="xr")
                rt = sb.tile([128, D], F32, tag="rt")
                xnv = xn_t.rearrange("p (h i two) -> p h i two", h=H, two=2)
                rtv = rt.rearrange("p (h d) -> p h d", h=H)
                rwv = rotw_t.rearrange("p (h d) -> p h d", h=H)
                nc.vector.tensor_mul(rtv[:, :, 0:32], xnv[:, :, :, 1], rwv[:, :, 0:32])
                nc.vector.tensor_mul(rtv[:, :, 32:64], xnv[:, :, :, 0], rwv[:, :, 32:64])
                nc.vector.tensor_mul(xr_t, xn_t, cosw_t)
                nc.vector.tensor_add(xr_t, xr_t, rt)
                if has_ropeb:
                    rb_t = sb.tile([128, D], F32, tag="rb")
                    nc.sync.dma_start(out=rb_t, in_=ropeb[tt * 128:(tt + 1) * 128, :])
                    nc.vector.tensor_add(xr_t, xr_t, rb_t)
                # transpose to D-major into SBUF (no DRAM round-trip)
                for dstT, src_tile in ((xrT, xr_t), (xnT, xn_t)):
                    ps = trps.tile([128, 512], F32, tag="tr")
                    for k in range(NK):
                        nc.tensor.transpose(ps[:, k * 128:(k + 1) * 128],
                                            src_tile[:, k * 128:(k + 1) * 128], ident)
                    nc.vector.tensor_copy(
                        dstT[:, :, t * 128:(t + 1) * 128],
                        ps.rearrange("p (k i) -> p k i", k=NK))
            # ---- P2a: q,k (D-major) + v (token-major) for local tokens ----
            tok0 = bb * SL
            for m in range(2 * D // 128):  # 8 output chunks over [q|k] dims
                ps = qvps.tile([128, SL], F32, tag="qk")
                for k in range(NK):
                    nc.tensor.matmul(ps, wqk_sb[:, k, m * 128:(m + 1) * 128],
                                     xrT[:, k, tok0:tok0 + SL],
                                     start=(k == 0), stop=(k == NK - 1))
                qk_sb = qv.tile([128, SL], BF16, tag="qksb")
                nc.vector.tensor_copy(qk_sb, ps)
                # rows 0:64 -> head 2m[+0], rows 64:128 -> head 2m+1 (q if m<4)
                plane = 0 if m < 4 else 1
                h0 = (m % 4) * 2
                for hh in range(2):
                    nc.sync.dma_start(
                        out=cc3_in[bb][h0 + hh, plane].rearrange("(p t) -> p t", p=Dh),
                        in_=qk_sb[hh * Dh:(hh + 1) * Dh, :])
            for tb in range(SL // 128):  # 4 token blocks, v token-major
                ps = qvps.tile([128, D], F32, tag="v")
                for k in range(NK):
                    nc.tensor.matmul(ps, xnT[:, k, tok0 + tb * 128:tok0 + (tb + 1) * 128],
                                     wv_sb[:, k, :],
                                     start=(k == 0), stop=(k == NK - 1))
                v_sb = qv.tile([128, D], BF16, tag="vsb")
                nc.vector.tensor_copy(v_sb, ps)
                # v_sb[p, h*64+d] -> cc3_in[h, 2, (tb*128+p)*64 + d]
                nc.sync.dma_start(
                    out=cc3_in[bb][:, 2].rearrange(
                        "h (t p d) -> h t p d", p=128, d=Dh)[:, tb]
                        .rearrange("h p d -> p h d"),
                    in_=v_sb.rearrange("p (h d) -> p h d", d=Dh))
            if not skip_cc and max_phase >= 2:
                nc.gpsimd.collective_compute(
                    "AllToAll", ALU.bypass,
                    ins=[cc3_in[bb].opt()], outs=[cc3_out[bb].opt()],
                    replica_groups=[list(range(W))],
                )

        _PHASE_MARKS.append(("P2a", nc.next_id()))

        # ====== P3: assemble my head's q,k,v from A2A output ======
        if max_phase >= 3:
            for bb in range(B):
                for j in range(W):
                    nc.sync.dma_start(
                        out=qT[:, bb, j * SL:(j + 1) * SL],
                        in_=cc3_out[bb][j, 0].rearrange("(p t) -> p t", p=Dh))
                    nc.sync.dma_start(
                        out=kT[:, bb, j * SL:(j + 1) * SL],
                        in_=cc3_out[bb][j, 1].rearrange("(p t) -> p t", p=Dh))
                    nc.sync.dma_start(
                        out=vS[:, bb, j * 4:(j + 1) * 4, 0:Dh],
                        in_=cc3_out[bb][j, 2].rearrange(
                            "(kt p d) -> p kt d", p=128, d=Dh))
                if has_bq or has_bk:
                    for j in range(W):
                        nc.vector.tensor_scalar_add(
                            qT[:, bb, j * SL:(j + 1) * SL],
                            qT[:, bb, j * SL:(j + 1) * SL], bq_sb)
                        nc.vector.tensor_scalar_add(
                            kT[:, bb, j * SL:(j + 1) * SL],
                            kT[:, bb, j * SL:(j + 1) * SL], bk_sb)
                if has_bv:
                    for kt in range(NS):
                        nc.vector.tensor_add(vS[:, bb, kt, 0:Dh],
                                             vS[:, bb, kt, 0:Dh], bv_bc)

        _PHASE_MARKS.append(("P3", nc.next_id()))

        # ====== P4: causal attention (software-pipelined) ======
        if max_phase >= 4:
          with ExitStack() as ctx:
            expp = ctx.enter_context(tc.tile_pool(name="p4e", bufs=4))
            nrm = ctx.enter_context(tc.tile_pool(name="p4n", bufs=3))
            scps = ctx.enter_context(tc.tile_pool(name="p4s", bufs=2, space="PSUM"))
            atps = ctx.enter_context(tc.tile_pool(name="p4a", bufs=2, space="PSUM"))
            bcps = ctx.enter_context(tc.tile_pool(name="p4b", bufs=2, space="PSUM"))
            # flat job list: (b, qb, pair)
            jobs = [(b, qb, p)
                    for b in range(B) for qb in range(8)
                    for p in range(2 * (qb + 1))]
            sc_ps = {}
            pa_cur = {}

            def emit_sc(job):
                b, qb, p = job
                q_rhs = qT[:, b, qb * 512:(qb + 1) * 512]
                ps = scps.tile([128, 1024], F32, tag="sc", name="sc_ps_t")
                for i in range(2):
                    kt = p * 2 + i
                    nc.tensor.matmul(ps[:, i * 512:(i + 1) * 512],
                                     kT[:, b, kt * 128:(kt + 1) * 128],
                                     q_rhs, start=True, stop=True)
                sc_ps[job] = ps

            def emit_pv(job):
                b, qb, p = job
                nkt = 4 * (qb + 1)
                ps = sc_ps.pop(job)
                if p == 0:
                    pa_cur[(b, qb)] = atps.tile([65, 512], F32, tag="pa",
                                                name="pa_t")
                pa = pa_cur[(b, qb)]
                ex = expp.tile([128, 1024], BF16, tag="ex", name="ex_t")
                nc.scalar.activation(out=ex, in_=ps, func=AF.Exp, scale=SCALE)
                for i in range(2):
                    kt = p * 2 + i
                    jm = kt - (nkt - 4)
                    if jm >= 0:
                        nc.vector.tensor_mul(ex[:, i * 512:(i + 1) * 512],
                                             ex[:, i * 512:(i + 1) * 512],
                                             masks[:, jm, :])
                    nc.tensor.matmul(pa, vS[:, b, kt, :],
                                     ex[:, i * 512:(i + 1) * 512],
                                     start=(kt == 0), stop=(kt == nkt - 1))
                if p == 2 * (qb + 1) - 1:
                    # normalization tail for this (b, qb)
                    pa = pa_cur.pop((b, qb))
                    pa_sb = nrm.tile([65, 512], F32, tag="pasb")
                    nc.vector.tensor_copy(pa_sb, pa)
                    rcp = nrm.tile([1, 512], F32, tag="rcp")
                    nc.vector.reciprocal(rcp, pa_sb[64:65, :])
                    # broadcast rcp across 64 partitions: ones^T @ rcp
                    bc = bcps.tile([Dh, 512], F32, tag="bc")
                    nc.tensor.matmul(bc, ones_row, rcp, start=True, stop=True)
                    nc.vector.tensor_mul(
                        attnT[:, b, qb * 512:(qb + 1) * 512],
                        pa_sb[0:64, :], bc)

            emit_sc(jobs[0])
            for idx, job in enumerate(jobs):
                if idx + 1 < len(jobs):
                    emit_sc(jobs[idx + 1])
                emit_pv(job)
                # ship + exchange each batch as soon as it completes
                b, qb, p = job
                if qb == 7 and p == 2 * (qb + 1) - 1:
                    nc.sync.dma_start(
                        out=cc2_in[b].rearrange("j d i -> d j i"),
                        in_=attnT[:, b, :].rearrange("d (j i) -> d j i", j=W))
                    if not skip_cc and max_phase >= 5:
                        nc.gpsimd.collective_compute(
                            "AllToAll", ALU.bypass,
                            ins=[cc2_in[b].opt()], outs=[cc2_out[b].opt()],
                            replica_groups=[list(range(W))],
                        )

        _PHASE_MARKS.append(("P4", nc.next_id()))

        # ========== P5: out_proj + residual + LN2 (+ transpose y) ==========
        if max_phase >= 6:
          with ExitStack() as ctx:
              sb = ctx.enter_context(tc.tile_pool(name="p5", bufs=3))
              small = ctx.enter_context(tc.tile_pool(name="p5s", bufs=4))
              ops = ctx.enter_context(tc.tile_pool(name="p5ps", bufs=2, space="PSUM"))
              trps = ctx.enter_context(tc.tile_pool(name="p5tr", bufs=2, space="PSUM"))
              for t in range(NT):
                  b, sc = t // (NT // B), t % (NT // B)
                  po = ops.tile([128, D], F32, tag="po")
                  for k in range(NK):
                      a_sb = sb.tile([128, 128], BF16, tag="a")
                      nc.sync.dma_start(
                          out=a_sb,
                          in_=cc2_out[b][2 * k:2 * k + 2, :,
                                         sc * 128:(sc + 1) * 128].rearrange(
                                             "e d i -> (e d) i"))
                      nc.tensor.matmul(po, a_sb, wo_sb[:, k, :],
                                       start=(k == 0), stop=(k == NK - 1))
                  s_t = sb.tile([128, D], F32, tag="s")
                  nc.sync.dma_start(out=s_t, in_=src_loc[t * 128:(t + 1) * 128, :])
                  o1 = out1[:, t, :]
                  nc.vector.tensor_add(o1, po, s_t)
                  if has_bo:
                      nc.vector.tensor_add(o1, o1, bo_bc)
                  # LN2 (affine folded into w1_t/b1p host-side)
                  rstd, nm = _layer_norm_stats(nc, small, o1, eps_sb)
                  y_t = sb.tile([128, D], F32, tag="y")
                  nc.vector.tensor_scalar(out=y_t, in0=o1, scalar1=rstd, scalar2=nm,
                                          op0=ALU.mult, op1=ALU.add)
                  ps = trps.tile([128, 512], F32, tag="tr")
                  for k in range(NK):
                      nc.tensor.transpose(ps[:, k * 128:(k + 1) * 128],
                                          y_t[:, k * 128:(k + 1) * 128], ident)
                  nc.vector.tensor_copy(
                      yT[:, :, t * 128:(t + 1) * 128],
                      ps.rearrange("p (k i) -> p k i", k=NK))

        _PHASE_MARKS.append(("P5", nc.next_id()))
        # ================= P6: FFN + final residual =================
        if max_phase >= 7:
          with ExitStack() as ctx:
              sb = ctx.enter_context(tc.tile_pool(name="p6", bufs=3))
              hps = ctx.enter_context(tc.tile_pool(name="p6h", bufs=2, space="PSUM"))
              o2ps = ctx.enter_context(tc.tile_pool(name="p6o", bufs=1, space="PSUM"))
              for th in range(2):
                  po2 = [o2ps.tile([128, D], F32, tag=f"po2_{tq}", name=f"po2_{tq}")
                         for tq in range(4)]
                  for m in range(NF):
                      ph = hps.tile([128, 512], F32, tag="ph")
                      for k in range(NK):
                          nc.tensor.matmul(ph, w1_sb[:, k, m * 128:(m + 1) * 128],
                                           yT[:, k, th * 512:(th + 1) * 512],
                                           start=(k == 0), stop=(k == NK - 1))
                      hT = sb.tile([128, 512], BF16, tag="hT")
                      nc.scalar.activation(out=hT, in_=ph,
                                           func=_GELU_OVERRIDE or AF.Gelu,
                                           bias=b1_sb[:, m:m + 1])
                      for tq in range(4):
                          nc.tensor.matmul(po2[tq], hT[:, tq * 128:(tq + 1) * 128],
                                           w2_sb[:, m, :],
                                           start=(m == 0), stop=(m == NF - 1))
                  for tq in range(4):
                      t = th * 4 + tq
                      fin = sb.tile([128, D], F32, tag="fin")
                      nc.vector.tensor_add(fin, po2[tq], out1[:, t, :])
                      if has_b2:
                          nc.vector.tensor_add(fin, fin, b2_bc)
                      nc.sync.dma_start(out=out_loc[t * 128:(t + 1) * 128, :], in_=fin)

        _PHASE_MARKS.append(("P6", nc.next_id()))
        if max_phase < 7:
            with tc.tile_pool(name="dummy", bufs=1) as dp:
                dt_ = dp.tile([128, D], F32)
                nc.vector.memset(dt_, 0.0)
                for i in range(TL // 128):
                    nc.sync.dma_start(out=out_loc[i * 128:(i + 1) * 128, :], in_=dt_)
    nc.compile()
    return nc


def _prep(inputs):
    src = np.asarray(inputs["src"], np.float32)
    cos = np.asarray(inputs["rotary_cos"], np.float32).reshape(S, Dh)
    sin = np.asarray(inputs["rotary_sin"], np.float32).reshape(S, Dh)
    ipw = np.asarray(inputs["in_proj_w"], np.float32)
    ipb = np.asarray(inputs["in_proj_b"], np.float32)
    opw = np.asarray(inputs["out_proj_w"], np.float32)
    opb = np.asarray(inputs["out_proj_b"], np.float32)
    w1 = np.asarray(inputs["w1"], np.float32)
    b1 = np.asarray(inputs["b1"], np.float32)
    w2 = np.asarray(inputs["w2"], np.float32)
    b2 = np.asarray(inputs["b2"], np.float32)
    ln1_w = np.asarray(inputs["ln1_w"], np.float32)
    ln1_b = np.asarray(inputs["ln1_b"], np.float32)
    ln2_w = np.asarray(inputs["ln2_w"], np.float32)
    ln2_b = np.asarray(inputs["ln2_b"], np.float32)

    cos_full = np.tile(cos, (1, H))            # [S, D]
    sin_full = np.tile(sin, (1, H))
    d = np.arange(D)
    jj = d % Dh
    hb = d - jj
    src2 = np.where(jj < 32, hb + 2 * jj + 1, hb + 2 * (jj - 32))
    sign = np.where(jj < 32, -1.0, 1.0).astype(np.float32)
    cosw_full = ln1_w[None, :] * cos_full
    rotw_full = (sign[None, :] * ln1_w[src2][None, :]) * sin_full
    ropeb_full = (ln1_b[None, :] * cos_full
                  + (sign[None, :] * ln1_b[src2][None, :]) * sin_full)

    wq, wk, wv = ipw[0:D], ipw[D:2 * D], ipw[2 * D:3 * D]
    bq, bk, bv = ipb[0:D], ipb[D:2 * D], ipb[2 * D:3 * D]
    wqk_t = np.ascontiguousarray(
        np.concatenate([wq.T, wk.T], axis=1)).astype(ml_dtypes.bfloat16)  # [D, 2D]
    wv_t = np.ascontiguousarray(ln1_w[:, None] * wv.T).astype(ml_dtypes.bfloat16)
    bv_full = np.ascontiguousarray(ln1_b @ wv.T + bv, np.float32)        # [D]
    w1_t = np.ascontiguousarray(ln2_w[:, None] * w1.T).astype(ml_dtypes.bfloat16)
    b1p = np.ascontiguousarray(ln2_b @ w1.T + b1, np.float32)
    wo_t = np.ascontiguousarray(opw.T).astype(ml_dtypes.bfloat16)
    w2_t = np.ascontiguousarray(w2.T).astype(ml_dtypes.bfloat16)

    flags = (
        bool(np.any(ropeb_full)), bool(np.any(bq)), bool(np.any(bk)),
        bool(np.any(bv) or np.any(ln1_b)), bool(np.any(opb)), bool(np.any(b2)),
    )

    in_maps = []
    for c in range(W):
        h0 = c * Dh
        m = {
            "src_loc": np.ascontiguousarray(
                src[SL * c:SL * (c + 1)].transpose(1, 0, 2).reshape(TL, D)),
            "cosw": np.ascontiguousarray(cosw_full[SL * c:SL * (c + 1)]),
            "rotw": np.ascontiguousarray(rotw_full[SL * c:SL * (c + 1)]),
            "wqk_t": wqk_t,
            "wv_t": wv_t,
            "bqk": np.concatenate([bq[h0:h0 + Dh], bk[h0:h0 + Dh]]),
            "bvh": bv_full[h0:h0 + Dh],
            "wo_t": wo_t,
            "bo": opb,
            "w1_t": w1_t,
            "b1p": b1p,
            "w2_t": w2_t,
            "b2": b2,
        }
        if flags[0]:
            m["ropeb"] = np.ascontiguousarray(ropeb_full[SL * c:SL * (c + 1)])
        in_maps.append(m)
    return in_maps, flags


def _get_nc(flags):
    if flags not in _NC_CACHE:
        _NC_CACHE[flags] = _build_nc(flags)
    return _NC_CACHE[flags]


def kernel(**inputs):
    in_maps, flags = _prep(inputs)
    nc = _get_nc(flags)
    res = run_bass_kernel_spmd(nc, in_maps, core_ids=list(range(W)))
    out = np.empty((S, B, D), np.float32)
    for c in range(W):
        ol = res.results[c]["out_loc"].reshape(B, SL, D)
        out[SL * c:SL * (c + 1)] = ol.transpose(1, 0, 2)
    return out


# revision 12
# speedup vs baseline: 1.0754x; 1.0754x over previous
"""Trainium2 Bass kernel: transformer encoder layer (S=4096,B=2,D=512,H=8,F=2048),
causal attention + RoPE, distributed over 8 NeuronCores.

Sharding (SPMD, v2 — AllToAll resharding, no AllGather):
  - LN1+RoPE: sequence-parallel (core c owns s in [512c, 512(c+1)), both batches)
  - QKV for ALL heads on LOCAL tokens (contraction over D, local xr/xn kept in SBUF)
  - AllToAll(qkv, 1.5MB/rank, per batch) -> core c gets head c's q,k,v for ALL s
  - causal attention: head-parallel (core c owns head c, full S, per batch)
  - AllToAll(attn, 0.5MB/rank, per batch) -> each core gets all heads for its tokens
  - out_proj + residual + LN2 + FFN: token-parallel (core c owns its s-slice)
LayerNorm affine params are folded into downstream weights host-side.
Softmax denominators come free from a ones-column appended to V; the
denominator reciprocal is broadcast across partitions with a PE outer product.
"""
import numpy as np
import ml_dtypes
from contextlib import ExitStack

import concourse.bass as bass
import concourse.tile as tile
from concourse import bacc, mybir
from concourse.bass_utils import run_bass_kernel_spmd
from concourse.masks import make_identity

F32 = mybir.dt.float32
F32R = mybir.dt.float32r
BF16 = mybir.dt.bfloat16
AF = mybir.ActivationFunctionType
ALU = mybir.AluOpType

S, B, D, H, Dh, F = 4096, 2, 512, 8, 64, 2048
W = 8                    # cores
SL = S // W              # 512 s-positions per core
TL = SL * B              # 1024 local tokens
EPS = 1e-5
SCALE = 1.0 / float(np.sqrt(Dh))  # 0.125

NT = TL // 128           # 8 local token tiles
NK = D // 128            # 4 contraction chunks over D
NF = F // 128            # 16 chunks over F
NS = S // 128            # 32 key tiles per batch

_NC_CACHE = {}
_PHASE_MARKS = []
_GELU_OVERRIDE = None  # set to AF.Identity in sim tests (CoreSim lacks Gelu)


def _layer_norm_stats(nc, pool, x_t, eps_sb):
    """Returns (rstd [128,1], negmean_rstd [128,1]) for rows of x_t."""
    stats = pool.tile([128, 6], F32, tag="st")
    nc.vector.bn_stats(out=stats, in_=x_t)
    mv = pool.tile([128, 2], F32, tag="mv")
    nc.vector.bn_aggr(out=mv, in_=stats)
    sd = pool.tile([128, 1], F32, tag="sd")
    nc.scalar.activation(out=sd, in_=mv[:, 1:2], func=AF.Sqrt, bias=eps_sb)
    rstd = pool.tile([128, 1], F32, tag="rs")
    nc.vector.reciprocal(out=rstd, in_=sd)
    nm = pool.tile([128, 1], F32, tag="nm")
    nc.vector.tensor_mul(nm, mv[:, 0:1], rstd)
    nc.vector.tensor_scalar_mul(nm, nm, -1.0)
    return rstd, nm


def _build_nc(flags, n_reps=1):
    """flags = (has_ropeb, has_bq, has_bk, has_bv, has_bo, has_b2)

    n_reps > 1 builds a timing variant with the body unrolled n_reps times
    (same I/O, idempotent) so device time can be read off the slope.
    """
    import os as _os
    has_ropeb, has_bq, has_bk, has_bv, has_bo, has_b2 = flags
    skip_cc = bool(int(_os.environ.get("K_SKIP_CC", "0")))
    max_phase = int(_os.environ.get("K_MAX_PHASE", "7"))
    nc = bacc.Bacc("TRN2", target_bir_lowering=False, debug=False, num_devices=W)

    # ---- I/O ----
    src_loc = nc.dram_tensor("src_loc", [TL, D], F32, kind="ExternalInput")
    cosw = nc.dram_tensor("cosw", [SL, D], F32, kind="ExternalInput")
    rotw = nc.dram_tensor("rotw", [SL, D], F32, kind="ExternalInput")
    ropeb = nc.dram_tensor("ropeb", [SL, D], F32, kind="ExternalInput") if has_ropeb else None
    wqk_t = nc.dram_tensor("wqk_t", [D, 2 * D], BF16, kind="ExternalInput")
    wv_t = nc.dram_tensor("wv_t", [D, D], BF16, kind="ExternalInput")
    bqk = nc.dram_tensor("bqk", [2 * Dh], F32, kind="ExternalInput")
    bvh = nc.dram_tensor("bvh", [Dh], F32, kind="ExternalInput")
    wo_t = nc.dram_tensor("wo_t", [D, D], BF16, kind="ExternalInput")
    bo = nc.dram_tensor("bo", [D], F32, kind="ExternalInput")
    w1_t = nc.dram_tensor("w1_t", [D, F], BF16, kind="ExternalInput")
    b1p = nc.dram_tensor("b1p", [F], F32, kind="ExternalInput")
    w2_t = nc.dram_tensor("w2_t", [F, D], BF16, kind="ExternalInput")
    b2 = nc.dram_tensor("b2", [D], F32, kind="ExternalInput")
    out_loc = nc.dram_tensor("out_loc", [TL, D], F32, kind="ExternalOutput")

    with tile.TileContext(nc) as tc:
      for _rep in range(n_reps):
       with ExitStack() as top:
        dram = top.enter_context(tc.tile_pool(name="dram", bufs=1, space="DRAM"))
        consts = top.enter_context(tc.tile_pool(name="consts", bufs=1))
        wpool = top.enter_context(tc.tile_pool(name="weights", bufs=1))
        act = top.enter_context(tc.tile_pool(name="act", bufs=1))

        # ---------- constants ----------
        ident = consts.tile([128, 128], F32)
        make_identity(nc, ident)
        # causal diag masks: masks[:, j, q] = 1.0 if q >= k + j*128 else 0.0
        masks = consts.tile([128, 4, 512], BF16)
        for j in range(4):
            nc.gpsimd.memset(masks[:, j, :], 1.0)
            nc.gpsimd.affine_select(
                out=masks[:, j, :], in_=masks[:, j, :],
                compare_op=ALU.is_ge, fill=0.0,
                base=-j * 128, channel_multiplier=-1, pattern=[[1, 512]],
            )
        eps_sb = consts.tile([128, 1], F32)
        nc.vector.memset(eps_sb, EPS)
        ones_row = consts.tile([1, Dh], F32)
        nc.vector.memset(ones_row, 1.0)
        bq_sb = consts.tile([Dh, 1], F32)
        nc.sync.dma_start(out=bq_sb, in_=bqk[0:Dh, None])
        bk_sb = consts.tile([Dh, 1], F32)
        nc.sync.dma_start(out=bk_sb, in_=bqk[Dh:2 * Dh, None])
        bv_bc = consts.tile([128, Dh], F32)
        if has_bv:
            bv_row = consts.tile([1, Dh], F32)
            nc.sync.dma_start(out=bv_row, in_=bvh[None, :])
            nc.gpsimd.partition_broadcast(bv_bc, bv_row)
        bo_bc = consts.tile([128, D], F32)
        if has_bo:
            bo_row = consts.tile([1, D], F32)
            nc.sync.dma_start(out=bo_row, in_=bo[None, :])
            nc.gpsimd.partition_broadcast(bo_bc, bo_row)
        b2_bc = consts.tile([128, D], F32)
        if has_b2:
            b2_row = consts.tile([1, D], F32)
            nc.sync.dma_start(out=b2_row, in_=b2[None, :])
            nc.gpsimd.partition_broadcast(b2_bc, b2_row)
        b1_sb = consts.tile([128, NF], F32)
        nc.sync.dma_start(out=b1_sb, in_=b1p.rearrange("(m p) -> p m", p=128))

        # ---------- persistent weights (SBUF) ----------
        wqk_sb = wpool.tile([128, NK, 2 * D], BF16)
        nc.sync.dma_start(out=wqk_sb, in_=wqk_t.rearrange("(k p) m -> p k m", p=128))
        wv_sb = wpool.tile([128, NK, D], BF16)
        nc.sync.dma_start(out=wv_sb, in_=wv_t.rearrange("(k p) m -> p k m", p=128))
        wo_sb = wpool.tile([128, NK, D], BF16)
        nc.sync.dma_start(out=wo_sb, in_=wo_t.rearrange("(k p) n -> p k n", p=128))
        w1_sb = wpool.tile([128, NK, F], BF16)
        nc.sync.dma_start(out=w1_sb, in_=w1_t.rearrange("(k p) n -> p k n", p=128))
        w2_sb = wpool.tile([128, NF, D], BF16)
        nc.sync.dma_start(out=w2_sb, in_=w2_t.rearrange("(m p) n -> p m n", p=128))

        # ---------- persistent activations (SBUF) ----------
        # q duplicated on both partition halves; k packed even/odd k-tiles on
        # partition halves -> score matmul pairs run concurrently (row tiling)
        qT = act.tile([128, B, S], BF16)       # my head's q (rows 64:128 = copy)
        kT = act.tile([128, B, S // 2], BF16)  # even k-tiles @0:64, odd @64:128
        vS = act.tile([128, B, NS, 65], BF16)  # token-major V + ones column
        nc.vector.memset(vS[:, :, :, 64:65], 1.0)
        attnT = act.tile([Dh, B, S], BF16)
        out1 = act.tile([128, NT, D], F32)     # post-attention residual stream
        yT = act.tile([128, NK, TL], BF16)     # LN2 output, D-major

        # collective buffers (per batch)
        cc3_in = [dram.tile([W, 3, Dh * SL], BF16, name=f"cc3_in_{b}") for b in range(B)]
        cc3_out = [dram.tile([W, 3, Dh * SL], BF16, name=f"cc3_out_{b}")
                   for b in range(B)]
        cc2_in = [dram.tile([W, Dh, SL], BF16, name=f"cc2_in_{b}") for b in range(B)]
        cc2_out = [dram.tile([W, Dh, SL], BF16, name=f"cc2_out_{b}")
                   for b in range(B)]

        _PHASE_MARKS.append(("consts", nc.next_id()))

        # ====== P1 + P2a per batch: LN1+RoPE, local QKV (all heads), A2A ======
        with ExitStack() as ctx:
          xln = ctx.enter_context(tc.tile_pool(name="xln", bufs=1))
          sb = ctx.enter_context(tc.tile_pool(name="p1", bufs=3))
          small = ctx.enter_context(tc.tile_pool(name="p1s", bufs=4))
          trps = ctx.enter_context(tc.tile_pool(name="p1ps", bufs=4, space="PSUM"))
          qv = ctx.enter_context(tc.tile_pool(name="p2", bufs=3))
          qvps = ctx.enter_context(tc.tile_pool(name="p2ps", bufs=2, space="PSUM"))
          xrT = xln.tile([128, NK, TL], BF16)   # D-major LN1+RoPE output
          xnT = xln.tile([128, NK, TL], BF16)   # D-major LN1 output
          for bb in range(B):
            # ---- P1: LN + RoPE + transpose for this batch's 4 tiles ----
            for tt in range(NT // B):
                t = bb * (NT // B) + tt
                s_t = sb.tile([128, D], F32, tag="s")
                nc.sync.dma_start(out=s_t, in_=src_loc[t * 128:(t + 1) * 128, :])
                rstd, nm = _layer_norm_stats(nc, small, s_t, eps_sb)
                xn_t = sb.tile([128, D], F32, tag="xn")
                nc.vector.tensor_scalar(
                    out=xn_t, in0=s_t, scalar1=rstd, scalar2=nm,
                    op0=ALU.mult, op1=ALU.add,
                )
                # RoPE (ln1 affine folded into cosw/rotw/ropeb host-side)
                cosw_t = sb.tile([128, D], F32, tag="cw")
                nc.sync.dma_start(out=cosw_t, in_=cosw[tt * 128:(tt + 1) * 128, :])
                rotw_t = sb.tile([128, D], F32, tag="rw")
                nc.sync.dma_start(out=rotw_t, in_=rotw[tt * 128:(tt + 1) * 128, :])
                xr_t = sb.tile([128, D], F32, tag="xr")
                rt = sb.tile([128, D], F32, tag="rt")
                xnv = xn_t.rearrange("p (h i two) -> p h i two", h=H, two=2)
                rtv = rt.rearrange("p (h d) -> p h d", h=H)
                rwv = rotw_t.rearrange("p (h d) -> p h d", h=H)
                nc.vector.tensor_mul(rtv[:, :, 0:32], xnv[:, :, :, 1], rwv[:, :, 0:32])
                nc.vector.tensor_mul(rtv[:, :, 32:64], xnv[:, :, :, 0], rwv[:, :, 32:64])
                nc.vector.tensor_mul(xr_t, xn_t, cosw_t)
                nc.vector.tensor_add(xr_t, xr_t, rt)
                if has_ropeb:
                    rb_t = sb.tile([128, D], F32, tag="rb")
                    nc.sync.dma_start(out=rb_t, in_=ropeb[tt * 128:(tt + 1) * 128, :])
                    nc.vector.tensor_add(xr_t, xr_t, rb_t)
                # transpose to D-major into SBUF (no DRAM round-trip)
                for dstT, src_tile in ((xrT, xr_t), (xnT, xn_t)):
                    ps = trps.tile([128, 512], F32, tag="tr")
                    for k in range(NK):
                        nc.tensor.transpose(ps[:, k * 128:(k + 1) * 128],
                                            src_tile[:, k * 128:(k + 1) * 128], ident)
                    nc.vector.tensor_copy(
                        dstT[:, :, t * 128:(t + 1) * 128],
                        ps.rearrange("p (k i) -> p k i", k=NK))
            # ---- P2a: q,k (D-major) + v (token-major) for local tokens ----
            tok0 = bb * SL
            for m in range(2 * D // 128):  # 8 output chunks over [q|k] dims
                ps = qvps.tile([128, SL], F32, tag="qk")
                for k in range(NK):
                    nc.tensor.matmul(ps, wqk_sb[:, k, m * 128:(m + 1) * 128],
                                     xrT[:, k, tok0:tok0 + SL],
                                     start=(k == 0), stop=(k == NK - 1))
                qk_sb = qv.tile([128, SL], BF16, tag="qksb")
                nc.vector.tensor_copy(qk_sb, ps)
                # rows 0:64 -> head 2m[+0], rows 64:128 -> head 2m+1 (q if m<4)
                plane = 0 if m < 4 else 1
                h0 = (m % 4) * 2
                for hh in range(2):
                    nc.sync.dma_start(
                        out=cc3_in[bb][h0 + hh, plane].rearrange("(p t) -> p t", p=Dh),
                        in_=qk_sb[hh * Dh:(hh + 1) * Dh, :])
            for tb in range(SL // 128):  # 4 token blocks, v token-major
                ps = qvps.tile([128, D], F32, tag="v")
                for k in range(NK):
                    nc.tensor.matmul(ps, xnT[:, k, tok0 + tb * 128:tok0 + (tb + 1) * 128],
                                     wv_sb[:, k, :],
                                     start=(k == 0), stop=(k == NK - 1))
                v_sb = qv.tile([128, D], BF16, tag="vsb")
                nc.vector.tensor_copy(v_sb, ps)
                # v_sb[p, h*64+d] -> cc3_in[h, 2, (tb*128+p)*64 + d]
                nc.sync.dma_start(
                    out=cc3_in[bb][:, 2].rearrange(
                        "h (t p d) -> h t p d", p=128, d=Dh)[:, tb]
                        .rearrange("h p d -> p h d"),
                    in_=v_sb.rearrange("p (h d) -> p h d", d=Dh))
            if not skip_cc and max_phase >= 2:
                nc.gpsimd.collective_compute(
                    "AllToAll", ALU.bypass,
                    ins=[cc3_in[bb].opt()], outs=[cc3_out[bb].opt()],
                    replica_groups=[list(range(W))],
                )

        _PHASE_MARKS.append(("P2a", nc.next_id()))

        # ====== P3: assemble my head's q,k,v from A2A output ======
        if max_phase >= 3:
            for bb in range(B):
                for j in range(W):
                    for half in range(2):
                        nc.sync.dma_start(
                            out=qT[half * Dh:(half + 1) * Dh, bb,
                                   j * SL:(j + 1) * SL],
                            in_=cc3_out[bb][j, 0].rearrange("(p t) -> p t", p=Dh))
                        # k-tiles jt=half,half+2 of rank j -> global 4j+jt
                        nc.sync.dma_start(
                            out=kT[half * Dh:(half + 1) * Dh, bb,
                                   2 * j * 128:(2 * j + 2) * 128],
                            in_=cc3_out[bb][j, 1].rearrange(
                                "(p jt i) -> p jt i", p=Dh, i=128)[:, half::2, :])
                    nc.sync.dma_start(
                        out=vS[:, bb, j * 4:(j + 1) * 4, 0:Dh],
                        in_=cc3_out[bb][j, 2].rearrange(
                            "(kt p d) -> p kt d", p=128, d=Dh))
                if has_bq or has_bk:
                    for j in range(W):
                        for half in range(2):
                            sl_q = qT[half * Dh:(half + 1) * Dh, bb,
                                      j * SL:(j + 1) * SL]
                            nc.vector.tensor_scalar_add(sl_q, sl_q, bq_sb)
                            sl_k = kT[half * Dh:(half + 1) * Dh, bb,
                                      2 * j * 128:(2 * j + 2) * 128]
                            nc.vector.tensor_scalar_add(sl_k, sl_k, bk_sb)
                if has_bv:
                    for kt in range(NS):
                        nc.vector.tensor_add(vS[:, bb, kt, 0:Dh],
                                             vS[:, bb, kt, 0:Dh], bv_bc)

        _PHASE_MARKS.append(("P3", nc.next_id()))

        # ====== P4: causal attention (software-pipelined) ======
        if max_phase >= 4:
          with ExitStack() as ctx:
            expp = ctx.enter_context(tc.tile_pool(name="p4e", bufs=4))
            nrm = ctx.enter_context(tc.tile_pool(name="p4n", bufs=3))
            scps = ctx.enter_context(tc.tile_pool(name="p4s", bufs=2, space="PSUM"))
            atps = ctx.enter_context(tc.tile_pool(name="p4a", bufs=2, space="PSUM"))
            bcps = ctx.enter_context(tc.tile_pool(name="p4b", bufs=2, space="PSUM"))
            # flat job list: (b, qb, pair)
            jobs = [(b, qb, p)
                    for b in range(B) for qb in range(8)
                    for p in range(2 * (qb + 1))]
            sc_ps = {}
            pa_cur = {}

            def emit_sc(job):
                # pair p covers k-tiles (2p, 2p+1), packed on partition halves
                # of kT -> the two matmuls occupy disjoint PE row groups and
                # stream concurrently.
                b, qb, p = job
                ps = scps.tile([128, 1024], F32, tag="sc", name="sc_ps_t")
                for i in range(2):
                    nc.tensor.matmul(ps[:, i * 512:(i + 1) * 512],
                                     kT[i * Dh:(i + 1) * Dh, b,
                                        p * 128:(p + 1) * 128],
                                     qT[i * Dh:(i + 1) * Dh, b,
                                        qb * 512:(qb + 1) * 512],
                                     start=True, stop=True)
                sc_ps[job] = ps

            def emit_pv(job):
                b, qb, p = job
                nkt = 4 * (qb + 1)
                ps = sc_ps.pop(job)
                if p == 0:
                    pa_cur[(b, qb)] = atps.tile([65, 512], F32, tag="pa",
                                                name="pa_t")
                pa = pa_cur[(b, qb)]
                ex = expp.tile([128, 1024], BF16, tag="ex", name="ex_t")
                nc.scalar.activation(out=ex, in_=ps, func=AF.Exp, scale=SCALE)
                for i in range(2):
                    kt = p * 2 + i
                    jm = kt - (nkt - 4)
                    if jm >= 0:
                        nc.vector.tensor_mul(ex[:, i * 512:(i + 1) * 512],
                                             ex[:, i * 512:(i + 1) * 512],
                                             masks[:, jm, :])
                    nc.tensor.matmul(pa, vS[:, b, kt, :],
                                     ex[:, i * 512:(i + 1) * 512],
                                     start=(kt == 0), stop=(kt == nkt - 1))
                if p == 2 * (qb + 1) - 1:
                    # normalization tail for this (b, qb)
                    pa = pa_cur.pop((b, qb))
                    pa_sb = nrm.tile([65, 512], F32, tag="pasb")
                    nc.vector.tensor_copy(pa_sb, pa)
                    rcp = nrm.tile([1, 512], F32, tag="rcp")
                    nc.vector.reciprocal(rcp, pa_sb[64:65, :])
                    # broadcast rcp across 64 partitions: ones^T @ rcp
                    bc = bcps.tile([Dh, 512], F32, tag="bc")
                    nc.tensor.matmul(bc, ones_row, rcp, start=True, stop=True)
                    nc.vector.tensor_mul(
                        attnT[:, b, qb * 512:(qb + 1) * 512],
                        pa_sb[0:64, :], bc)

            emit_sc(jobs[0])
            for idx, job in enumerate(jobs):
                if idx + 1 < len(jobs):
                    emit_sc(jobs[idx + 1])
                emit_pv(job)
                # ship + exchange each batch as soon as it completes
                b, qb, p = job
                if qb == 7 and p == 2 * (qb + 1) - 1:
                    nc.sync.dma_start(
                        out=cc2_in[b].rearrange("j d i -> d j i"),
                        in_=attnT[:, b, :].rearrange("d (j i) -> d j i", j=W))
                    if not skip_cc and max_phase >= 5:
                        nc.gpsimd.collective_compute(
                            "AllToAll", ALU.bypass,
                            ins=[cc2_in[b].opt()], outs=[cc2_out[b].opt()],
                            replica_groups=[list(range(W))],
                        )

        _PHASE_MARKS.append(("P4", nc.next_id()))

        # ==== P5+P6 interleaved per batch half: out_proj+LN2 then FFN ====
        # PSUM budget: p5ps 1 + p5tr 1 + p6h 2 + p6o 4 = 8 banks exactly.
        if max_phase >= 6:
          with ExitStack() as ctx:
              sb = ctx.enter_context(tc.tile_pool(name="p5", bufs=3))
              small = ctx.enter_context(tc.tile_pool(name="p5s", bufs=4))
              ops = ctx.enter_context(tc.tile_pool(name="p5ps", bufs=1, space="PSUM"))
              trps = ctx.enter_context(tc.tile_pool(name="p5tr", bufs=1, space="PSUM"))
              sb6 = ctx.enter_context(tc.tile_pool(name="p6", bufs=3))
              hps = ctx.enter_context(tc.tile_pool(name="p6h", bufs=2, space="PSUM"))
              o2ps = ctx.enter_context(tc.tile_pool(name="p6o", bufs=1, space="PSUM"))

              def emit_p5(b):
                  for sc in range(NT // B):
                      t = b * (NT // B) + sc
                      # one DMA: all heads x 128 tokens, grouped (e d) x k x i
                      a_sb = sb.tile([128, NK, 128], BF16, tag="a")
                      nc.sync.dma_start(
                          out=a_sb,
                          in_=cc2_out[b][:, :, sc * 128:(sc + 1) * 128].rearrange(
                              "(k e) d i -> (e d) k i", e=2))
                      po = ops.tile([128, D], F32, tag="po")
                      for k in range(NK):
                          nc.tensor.matmul(po, a_sb[:, k, :], wo_sb[:, k, :],
                                           start=(k == 0), stop=(k == NK - 1))
                      s_t = sb.tile([128, D], F32, tag="s")
                      nc.sync.dma_start(out=s_t, in_=src_loc[t * 128:(t + 1) * 128, :])
                      o1 = out1[:, t, :]
                      nc.vector.tensor_add(o1, po, s_t)
                      if has_bo:
                          nc.vector.tensor_add(o1, o1, bo_bc)
                      # LN2 (affine folded into w1_t/b1p host-side)
                      rstd, nm = _layer_norm_stats(nc, small, o1, eps_sb)
                      y_t = sb.tile([128, D], F32, tag="y")
                      nc.vector.tensor_scalar(out=y_t, in0=o1, scalar1=rstd,
                                              scalar2=nm, op0=ALU.mult, op1=ALU.add)
                      ps = trps.tile([128, 512], F32, tag="tr")
                      for k in range(NK):
                          nc.tensor.transpose(ps[:, k * 128:(k + 1) * 128],
                                              y_t[:, k * 128:(k + 1) * 128], ident)
                      nc.vector.tensor_copy(
                          yT[:, :, t * 128:(t + 1) * 128],
                          ps.rearrange("p (k i) -> p k i", k=NK))

              def emit_p6(th):
                  po2 = [o2ps.tile([128, D], F32, tag=f"po2_{tq}", name=f"po2_{tq}")
                         for tq in range(4)]
                  for m in range(NF):
                      ph = hps.tile([128, 512], F32, tag="ph")
                      for k in range(NK):
                          nc.tensor.matmul(ph, w1_sb[:, k, m * 128:(m + 1) * 128],
                                           yT[:, k, th * 512:(th + 1) * 512],
                                           start=(k == 0), stop=(k == NK - 1))
                      hT = sb6.tile([128, 512], BF16, tag="hT")
                      nc.scalar.activation(out=hT, in_=ph,
                                           func=_GELU_OVERRIDE or AF.Gelu,
                                           bias=b1_sb[:, m:m + 1])
                      for tq in range(4):
                          nc.tensor.matmul(po2[tq], hT[:, tq * 128:(tq + 1) * 128],
                                           w2_sb[:, m, :],
                                           start=(m == 0), stop=(m == NF - 1))
                  for tq in range(4):
                      t = th * 4 + tq
                      fin = sb6.tile([128, D], F32, tag="fin")
                      nc.vector.tensor_add(fin, po2[tq], out1[:, t, :])
                      if has_b2:
                          nc.vector.tensor_add(fin, fin, b2_bc)
                      nc.sync.dma_start(out=out_loc[t * 128:(t + 1) * 128, :],
                                        in_=fin)

              for b in range(B):
                  emit_p5(b)
                  if max_phase >= 7:
                      emit_p6(b)

        _PHASE_MARKS.append(("P6", nc.next_id()))
        if max_phase < 7:
            with tc.tile_pool(name="dummy", bufs=1) as dp:
                dt_ = dp.tile([128, D], F32)
                nc.vector.memset(dt_, 0.0)
                for i in range(TL // 128):
                    nc.sync.dma_start(out=out_loc[i * 128:(i + 1) * 128, :], in_=dt_)
    nc.compile()
    return nc


def _prep(inputs):
    src = np.asarray(inputs["src"], np.float32)
    cos = np.asarray(inputs["rotary_cos"], np.float32).reshape(S, Dh)
    sin = np.asarray(inputs["rotary_sin"], np.float32).reshape(S, Dh)
    ipw = np.asarray(inputs["in_proj_w"], np.float32)
    ipb = np.asarray(inputs["in_proj_b"], np.float32)
    opw = np.asarray(inputs["out_proj_w"], np.float32)
    opb = np.asarray(inputs["out_proj_b"], np.float32)
    w1 = np.asarray(inputs["w1"], np.float32)
    b1 = np.asarray(inputs["b1"], np.float32)
    w2 = np.asarray(inputs["w2"], np.float32)
    b2 = np.asarray(inputs["b2"], np.float32)
    ln1_w = np.asarray(inputs["ln1_w"], np.float32)
    ln1_b = np.asarray(inputs["ln1_b"], np.float32)
    ln2_w = np.asarray(inputs["ln2_w"], np.float32)
    ln2_b = np.asarray(inputs["ln2_b"], np.float32)

    cos_full = np.tile(cos, (1, H))            # [S, D]
    sin_full = np.tile(sin, (1, H))
    d = np.arange(D)
    jj = d % Dh
    hb = d - jj
    src2 = np.where(jj < 32, hb + 2 * jj + 1, hb + 2 * (jj - 32))
    sign = np.where(jj < 32, -1.0, 1.0).astype(np.float32)
    cosw_full = ln1_w[None, :] * cos_full
    rotw_full = (sign[None, :] * ln1_w[src2][None, :]) * sin_full
    ropeb_full = (ln1_b[None, :] * cos_full
                  + (sign[None, :] * ln1_b[src2][None, :]) * sin_full)

    wq, wk, wv = ipw[0:D], ipw[D:2 * D], ipw[2 * D:3 * D]
    bq, bk, bv = ipb[0:D], ipb[D:2 * D], ipb[2 * D:3 * D]
    wqk_t = np.ascontiguousarray(
        np.concatenate([wq.T, wk.T], axis=1)).astype(ml_dtypes.bfloat16)  # [D, 2D]
    wv_t = np.ascontiguousarray(ln1_w[:, None] * wv.T).astype(ml_dtypes.bfloat16)
    bv_full = np.ascontiguousarray(ln1_b @ wv.T + bv, np.float32)        # [D]
    w1_t = np.ascontiguousarray(ln2_w[:, None] * w1.T).astype(ml_dtypes.bfloat16)
    b1p = np.ascontiguousarray(ln2_b @ w1.T + b1, np.float32)
    wo_t = np.ascontiguousarray(opw.T).astype(ml_dtypes.bfloat16)
    w2_t = np.ascontiguousarray(w2.T).astype(ml_dtypes.bfloat16)

    flags = (
        bool(np.any(ropeb_full)), bool(np.any(bq)), bool(np.any(bk)),
        bool(np.any(bv) or np.any(ln1_b)), bool(np.any(opb)), bool(np.any(b2)),
    )

    in_maps = []
    for c in range(W):
        h0 = c * Dh
        m = {
            "src_loc": np.ascontiguousarray(
                src[SL * c:SL * (c + 1)].transpose(1, 0, 2).reshape(TL, D)),
            "cosw": np.ascontiguousarray(cosw_full[SL * c:SL * (c + 1)]),
            "rotw": np.ascontiguousarray(rotw_full[SL * c:SL * (c + 1)]),
            "wqk_t": wqk_t,
            "wv_t": wv_t,
            "bqk": np.concatenate([bq[h0:h0 + Dh], bk[h0:h0 + Dh]]),
            "bvh": bv_full[h0:h0 + Dh],
            "wo_t": wo_t,
            "bo": opb,
            "w1_t": w1_t,
            "b1p": b1p,
            "w2_t": w2_t,
            "b2": b2,
        }
        if flags[0]:
            m["ropeb"] = np.ascontiguousarray(ropeb_full[SL * c:SL * (c + 1)])
        in_maps.append(m)
    return in_maps, flags


def _get_nc(flags):
    if flags not in _NC_CACHE:
        _NC_CACHE[flags] = _build_nc(flags)
    return _NC_CACHE[flags]


def kernel(**inputs):
    in_maps, flags = _prep(inputs)
    nc = _get_nc(flags)
    res = run_bass_kernel_spmd(nc, in_maps, core_ids=list(range(W)))
    out = np.empty((S, B, D), np.float32)
    for c in range(W):
        ol = res.results[c]["out_loc"].reshape(B, SL, D)
        out[SL * c:SL * (c + 1)] = ol.transpose(1, 0, 2)
    return out
